# revision 20
# baseline (speedup 1.0000x reference)
"""Multi-head attention block on 8 TRN2 NeuronCores.

Sharding: core c -> (batch b = c//2, head-group hg = c%2).
Each core computes QKV projections for its 8 heads over its batch,
attention (bf16 QK^T with row-tiled concurrent head pairs, exp on ACT,
bf16 A@V with a col-packed ones-matmul producing replicated row-sums),
and a bf16 output projection of its head-group's channels. Pairs of
cores (same batch) combine partial projections with ReduceScatter
collectives; the host concatenates the 8 per-core output shards.

v2 schedule: single flat 256-task loop (4 q-blocks x 4 head-pairs x
16 k-blocks) with the exp pipeline as the pacing engine. All
projection work (K/V/Q/proj) is emitted through a per-q-block service
queue with just-in-time need-by indices so PE fills the exp-shadow
without starving ACT. Prologue uses batched priority DMAs (critical
1.5MB first) so the first exp fires ~10us in. Norms are pair-combined
(one reciprocal per head-pair) and chunked; qb3's ReduceScatter is
chunked per 128-row block to shorten the serial tail.
"""

import sys

if "/opt/trn_rl_repo" not in sys.path:
    sys.path.insert(0, "/opt/trn_rl_repo")

import numpy as np
import ml_dtypes

N_CORES = 8
B, T, DIM = 4, 2048, 1024
H_TOT, HD = 16, 64
HPC = H_TOT // 2          # heads per core (2 head-groups)
DQ = HPC * HD             # 512: per-core q/k/v width
SCALE = HD ** -0.5
KB_T = T // 128           # 16 k-blocks over sequence
KB_C = DIM // 128         # 8 k-blocks over channel dim

_CACHE = {}


def _build():
    import concourse.bass as bass
    import concourse.tile as tile
    from concourse import bacc, mybir

    F32 = mybir.dt.float32
    BF16 = mybir.dt.bfloat16
    AF = mybir.ActivationFunctionType

    nc = bacc.Bacc("TRN2", target_bir_lowering=False, debug=False,
                   num_devices=N_CORES)

    x_t = nc.dram_tensor("x_t", [DIM, T], BF16, kind="ExternalInput").ap()
    w_q_h = nc.dram_tensor("w_q_h", [128, 4, KB_C, 128], BF16,
                           kind="ExternalInput").ap()
    w_k_h = nc.dram_tensor("w_k_h", [128, 4, KB_C, 128], BF16,
                           kind="ExternalInput").ap()
    w_v_h = nc.dram_tensor("w_v_h", [128, KB_C, DQ], BF16,
                           kind="ExternalInput").ap()
    w_p_h = nc.dram_tensor("w_p_h", [128, 4, DIM], BF16,
                           kind="ExternalInput").ap()
    b_qkv = nc.dram_tensor("b_qkv_s", [3 * DQ], F32, kind="ExternalInput").ap()
    b_proj = nc.dram_tensor("b_proj_h", [DIM], F32, kind="ExternalInput").ap()
    out = nc.dram_tensor("out", [T // 2, DIM], BF16, kind="ExternalOutput").ap()
    # per-column-half planes so chunked ReduceScatter inputs stay contiguous
    partial_c = [nc.dram_tensor(f"partial{cb}", [T, DIM // 2], BF16).ap()
                 for cb in range(2)]
    rs_out_c = [nc.dram_tensor(f"rs_out{cb}", [T // 2, DIM // 2], BF16).ap()
                for cb in range(2)]
    partial_f = nc.dram_tensor("partial_f", [512, DIM], BF16).ap()
    rs_out_f = nc.dram_tensor("rs_out_f", [256, DIM], BF16).ap()

    groups = [[0, 1], [2, 3], [4, 5], [6, 7]]

    def bcast_ap(src_ap, parts):
        # partition-broadcast read of a 1-D DRAM row
        return bass.AP(tensor=src_ap.tensor, offset=src_ap.offset,
                       ap=[[0, parts]] + list(src_ap.ap))

    with tile.TileContext(nc) as tc:
        with (
            tc.tile_pool(name="persist", bufs=1) as pp,
        ):
            k_sb = pp.tile([128, 4, T], BF16)
            v_sb = pp.tile([128, KB_T, HPC, 2 * HD], BF16)
            wq_sb = pp.tile([128, 4, KB_C, 128], BF16)
            wk_c = pp.tile([128, 4, KB_C, 128], BF16)
            wv_c = pp.tile([128, KB_C, DQ], BF16)
            x_sb = pp.tile([128, KB_C, T], BF16)
            bqkv_sb = pp.tile([128, 12], F32)
            bv_sb = pp.tile([128, DQ], F32)

            nc.vector.memset(v_sb[:, :, :, HD:2 * HD], 1.0)

            with (
                tc.tile_pool(name="zb", bufs=1) as zb,
                tc.tile_pool(name="qsl", bufs=2) as qslp,
                tc.tile_pool(name="zpool", bufs=3) as zpool,
                tc.tile_pool(name="apool", bufs=5) as apool,
                tc.tile_pool(name="small", bufs=8) as small,
                tc.tile_pool(name="opool", bufs=4) as opool,
                tc.tile_pool(name="psS", bufs=2, space="PSUM") as pss,
                tc.tile_pool(name="psZ", bufs=2, space="PSUM") as psz,
                tc.tile_pool(name="psP", bufs=2, space="PSUM") as psp,
            ):
                wp_sb = zb.tile([128, 4, DIM], BF16)
                bp_sb = zb.tile([128, DIM], F32)
                warm1 = zb.tile([1, 4], F32)

                q_tiles = {}
                z_tiles = {}
                s_tiles = {}
                z_ps = {}
                zv_t = {}
                zs_t = {}
                qproj_ps = {}

                # ---------- emit helpers -------------------------------
                def emit_qproj_q(qb, m, h):
                    # 2-MM quarter of the q-projection chain
                    key = (qb, m)
                    if key not in qproj_ps:
                        if qb not in q_tiles:
                            q_tiles[qb] = qslp.tile([128, 4, 512], BF16,
                                                    tag="q", name=f"qt{qb}")
                        qproj_ps[key] = psp.tile([128, 512], F32, tag="pj",
                                                 name=f"qp{qb}_{m}")
                    ps = qproj_ps[key]
                    for kb in range(2 * h, 2 * h + 2):
                        nc.tensor.matmul(
                            ps[:],
                            wq_sb[:, m, kb, :],
                            x_sb[:, kb, 512 * qb:512 * (qb + 1)],
                            start=(kb == 0), stop=(kb == KB_C - 1))
                    if h == 3:
                        nc.vector.tensor_scalar_add(
                            out=q_tiles[qb][:, m, :],
                            in0=ps[:],
                            scalar1=bqkv_sb[:, m:m + 1])
                        del qproj_ps[key]

                def emit_qproj_half(qb, m, h):
                    emit_qproj_q(qb, m, 2 * h)
                    emit_qproj_q(qb, m, 2 * h + 1)

                def emit_k_half(m, half):
                    tcol = 512 * half
                    psx = psp.tile([128, 512], F32, tag="pj",
                                   name=f"kp{m}_{half}")
                    for kb in range(KB_C):
                        nc.tensor.matmul(
                            psx[:],
                            wk_c[:, m, kb, :],
                            x_sb[:, kb, tcol:tcol + 512],
                            start=(kb == 0), stop=(kb == KB_C - 1))
                    nc.vector.tensor_scalar_add(
                        out=k_sb[:, m, tcol:tcol + 512],
                        in0=psx[:], scalar1=bqkv_sb[:, 4 + m:5 + m])

                def emit_v_unit(tb):
                    ps = psp.tile([128, DQ], F32, tag="pj", name=f"vps{tb}")
                    for kb in range(KB_C):
                        nc.tensor.matmul(
                            ps[:],
                            x_sb[:, kb, 128 * tb:128 * (tb + 1)],
                            wv_c[:, kb, :],
                            start=(kb == 0), stop=(kb == KB_C - 1))
                    nc.vector.tensor_add(
                        v_sb[:, tb, :, 0:HD],
                        ps[:].rearrange("p (h d) -> p h d", h=HPC),
                        bv_sb[:].rearrange("p (h d) -> p h d", h=HPC))

                def emit_qk_pair(qb, hp, kb, s):
                    qt = q_tiles[qb]
                    kc = 128 * kb
                    nc.tensor.matmul(
                        s[:, 0, :],
                        k_sb[0:64, hp, kc:kc + 128],
                        qt[0:64, hp, :],
                        start=True, stop=True)
                    nc.tensor.matmul(
                        s[:, 1, :],
                        k_sb[64:128, hp, kc:kc + 128],
                        qt[64:128, hp, :],
                        start=True, stop=True)

                def emit_av_pair(hp, kb, z0, z1, a):
                    st = (kb == 0)
                    sp = (kb == KB_T - 1)
                    nc.tensor.matmul(
                        z0[:], v_sb[:, kb, 2 * hp, :],
                        a[:, 0, :], start=st, stop=sp)
                    nc.tensor.matmul(
                        z1[:], v_sb[:, kb, 2 * hp + 1, :],
                        a[:, 1, :], start=st, stop=sp)

                def emit_zvzs(qb, hp):
                    # split accumulators into a value-pair tile and a
                    # sums-pair tile (head 2hp -> parts 0:64, 2hp+1 -> 64:128)
                    zv = small.tile([128, 512], F32, tag="zv",
                                    name=f"zv{qb}_{hp}")
                    zs = small.tile([128, 512], F32, tag="zs",
                                    name=f"zs{qb}_{hp}")
                    nc.vector.tensor_copy(zv[0:64, :], z_ps[2 * hp][0:64, :])
                    nc.vector.tensor_copy(zv[64:128, :],
                                          z_ps[2 * hp + 1][0:64, :])
                    nc.vector.tensor_copy(zs[0:64, :],
                                          z_ps[2 * hp][64:128, :])
                    nc.vector.tensor_copy(zs[64:128, :],
                                          z_ps[2 * hp + 1][64:128, :])
                    zv_t[hp] = zv
                    zs_t[hp] = zs

                def emit_norm_chunk(qb, hp, ch):
                    # one 256-col chunk of the pair-combined normalize
                    c0, c1 = 256 * ch, 256 * (ch + 1)
                    rinv = small.tile([128, 256], F32, tag="rinv",
                                      name=f"ri{qb}_{hp}_{ch}")
                    nc.vector.reciprocal(rinv[:], zs_t[hp][:, c0:c1])
                    nc.gpsimd.tensor_mul(
                        z_tiles[qb][:, hp, c0:c1],
                        zv_t[hp][:, c0:c1], rinv[:])

                proj_ps = {}

                def emit_proj_half(qb, tb4, cb, h):
                    key = (qb, tb4, cb)
                    if key not in proj_ps:
                        proj_ps[key] = psp.tile([128, 512], F32, tag="pj",
                                                name=f"pj{qb}_{tb4}_{cb}")
                    zt = z_tiles[qb]
                    for m in range(2 * h, 2 * h + 2):
                        nc.tensor.matmul(
                            proj_ps[key][:],
                            zt[:, m, 128 * tb4:128 * (tb4 + 1)],
                            wp_sb[:, m, 512 * cb:512 * (cb + 1)],
                            start=(m == 0), stop=(m == 3))
                    if h == 1:
                        _finish_proj(qb, tb4, cb, proj_ps.pop(key))

                def emit_proj_unit(qb, tb4, cb):
                    emit_proj_half(qb, tb4, cb, 0)
                    emit_proj_half(qb, tb4, cb, 1)

                def _finish_proj(qb, tb4, cb, ppj):
                    t0 = 512 * qb + 128 * tb4
                    o = opool.tile([128, 512], BF16, tag="o")
                    nc.vector.tensor_add(
                        o[:], ppj[:], bp_sb[:, 512 * cb:512 * (cb + 1)])
                    if qb == 3:  # noqa: tail plane
                        nc.sync.dma_start(
                            out=partial_f[128 * tb4:128 * (tb4 + 1),
                                          512 * cb:512 * (cb + 1)],
                            in_=o[:])
                    else:
                        nc.sync.dma_start(
                            out=partial_c[cb][t0:t0 + 128, :],
                            in_=o[:])

                def emit_rs(qb, cb):
                    # column-half chunks: flat split of [512 rows, 512 cols]
                    # hands member p the contiguous 256-row half it owns
                    r0, r1 = 512 * qb, 512 * (qb + 1)
                    o0, o1 = 256 * qb, 256 * (qb + 1)
                    nc.gpsimd.collective_compute(
                        "ReduceScatter",
                        mybir.AluOpType.add,
                        ins=[partial_c[cb][r0:r1, :]],
                        outs=[rs_out_c[cb][o0:o1, :]],
                        replica_groups=groups,
                    )
                    nc.sync.dma_start(
                        out=out[o0:o1, 512 * cb:512 * (cb + 1)],
                        in_=rs_out_c[cb][o0:o1, :])

                def emit_rs_full():
                    nc.gpsimd.collective_compute(
                        "ReduceScatter",
                        mybir.AluOpType.add,
                        ins=[partial_f[:]],
                        outs=[rs_out_f[:]],
                        replica_groups=groups,
                    )
                    nc.sync.dma_start(out=out[768:1024, :], in_=rs_out_f[:])

                # ---------- prologue -----------------------------------
                # warm the exp table set while DMAs stream
                nc.vector.memset(warm1[:], 0.0)
                nc.scalar.activation(out=warm1[:], in_=warm1[:],
                                     func=AF.Exp, scale=1.0)

                nc.sync.dma_start(out=bqkv_sb,
                                  in_=b_qkv.rearrange("(m p) -> p m", p=128))
                nc.sync.dma_start(out=bv_sb,
                                  in_=bcast_ap(b_qkv[2 * DQ:3 * DQ], 128))
                x_r = x_t.rearrange("(kb p) t -> p kb t", p=128)
                nc.sync.dma_start(out=x_sb[:, 0:4, 0:512],
                                  in_=x_r[:, 0:4, 0:512])
                nc.sync.dma_start(out=x_sb[:, 4:8, 0:512],
                                  in_=x_r[:, 4:8, 0:512])
                nc.sync.dma_start(out=wk_c[:, 0], in_=w_k_h[:, 0])
                nc.sync.dma_start(out=wq_sb[:, 0], in_=w_q_h[:, 0])
                nc.sync.dma_start(out=wv_c, in_=w_v_h)

                emit_k_half(0, 0)
                emit_qproj_half(0, 0, 0)
                emit_qproj_half(0, 0, 1)

                nc.sync.dma_start(out=x_sb[:, :, 512:1024],
                                  in_=x_r[:, :, 512:1024])
                nc.sync.dma_start(out=x_sb[:, :, 1024:1536],
                                  in_=x_r[:, :, 1024:1536])
                nc.sync.dma_start(out=x_sb[:, :, 1536:2048],
                                  in_=x_r[:, :, 1536:2048])
                nc.sync.dma_start(out=wk_c[:, 1:4], in_=w_k_h[:, 1:4])
                nc.sync.dma_start(out=wq_sb[:, 1:4], in_=w_q_h[:, 1:4])
                nc.sync.dma_start(out=wp_sb, in_=w_p_h)
                nc.sync.dma_start(out=bp_sb, in_=bcast_ap(b_proj[:], 128))

                # ---------- service queues -----------------------------
                def build_svc(qb):
                    svc = []
                    if qb == 0:
                        for tb in range(2, KB_T):
                            svc.append((max(0, tb - 1),
                                        lambda tb=tb: emit_v_unit(tb)))
                        for m in range(4):
                            for half in range(4):
                                if m == 0 and half == 0:
                                    continue
                                svc.append(
                                    (max(0, 16 * m + 4 * half - 2),
                                     lambda m=m, half=half:
                                     emit_k_half(m, half)))
                        for m in range(1, 4):
                            for h in range(4):
                                svc.append((16 * m - 6 + h,
                                            lambda m=m, h=h:
                                            emit_qproj_q(0, m, h)))
                        for h in range(4):
                            svc.append((52 + 2 * h,
                                        lambda h=h: emit_qproj_q(1, 0, h)))
                    else:
                        for m in range(1, 4):
                            for h in range(4):
                                svc.append((16 * m - 6 + h,
                                            lambda m=m, h=h:
                                            emit_qproj_q(qb, m, h)))
                        if qb < 3:
                            for h in range(4):
                                svc.append((52 + 2 * h,
                                            lambda h=h:
                                            emit_qproj_q(qb + 1, 0, h)))
                        for u in range(8):
                            cb, tb4 = u // 4, u % 4
                            for h in range(2):
                                svc.append(
                                    (4 + 6 * u + 2 * h,
                                     lambda tb4=tb4, cb=cb, h=h:
                                     emit_proj_half(qb - 1, tb4, cb, h)))
                        svc.append((26, lambda: emit_rs(qb - 1, 0)))
                        svc.append((52, lambda: emit_rs(qb - 1, 1)))
                    svc.sort(key=lambda x: x[0])
                    return svc

                # ---------- main loop ----------------------------------
                NT = 256
                z_tiles[0] = zpool.tile([128, 4, 512], BF16, tag="z",
                                        name="zt0")
                s_tiles[0] = pss.tile([128, 2, 512], F32, tag="s", name="s0")
                emit_qk_pair(0, 0, 0, s_tiles[0])
                emit_v_unit(0)
                emit_v_unit(1)
                svc = build_svc(0)
                svc_i = 0

                for g in range(NT):
                    qb, idx = g // 64, g % 64
                    hp, kb = idx // 16, idx % 16

                    if idx == 0 and qb > 0:
                        svc = build_svc(qb)
                        svc_i = 0

                    # next task's QK (cross-boundary safe)
                    if g + 1 < NT:
                        nqb = (g + 1) // 64
                        if nqb != qb:
                            z_tiles[nqb] = zpool.tile([128, 4, 512], BF16,
                                                      tag="z", name=f"zt{nqb}")
                        nhp, nkb = ((g + 1) % 64) // 16, ((g + 1) % 64) % 16
                        s_tiles[g + 1] = pss.tile([128, 2, 512], F32, tag="s",
                                                  name=f"s{g + 1}")
                        emit_qk_pair(nqb, nhp, nkb, s_tiles[g + 1])

                    # exp of current task
                    a = apool.tile([128, 2, 512], BF16, tag="a",
                                   name=f"a{g}")
                    nc.scalar.activation(out=a[:], in_=s_tiles[g][:],
                                         func=AF.Exp, scale=SCALE)
                    del s_tiles[g]

                    # service work fills the exp shadow
                    while svc_i < len(svc) and svc[svc_i][0] <= idx:
                        svc[svc_i][1]()
                        svc_i += 1

                    # A@V accumulation
                    if kb == 0:
                        z_ps[2 * hp] = psz.tile([128, 512], F32, tag="z",
                                                name=f"zp{qb}_{2 * hp}")
                        z_ps[2 * hp + 1] = psz.tile(
                            [128, 512], F32, tag="z",
                            name=f"zp{qb}_{2 * hp + 1}")
                    emit_av_pair(hp, kb, z_ps[2 * hp], z_ps[2 * hp + 1], a)

                    # norms of the previous pair, chunked over 2 tasks
                    if hp >= 1 and kb in (2, 3):
                        emit_norm_chunk(qb, hp - 1, kb - 2)
                    if kb == KB_T - 1:
                        emit_zvzs(qb, hp)
                        if hp == 3:
                            emit_norm_chunk(qb, 3, 0)
                            emit_norm_chunk(qb, 3, 1)

                # ---------- tail: qb3 proj + one full-width RS ---------
                for tb4 in range(4):
                    emit_proj_unit(3, tb4, 0)
                    emit_proj_unit(3, tb4, 1)
                emit_rs_full()

    nc.compile()
    return nc


def _get_nc():
    if "nc" not in _CACHE:
        _CACHE["nc"] = _build()
    return _CACHE["nc"]


def _make_in_maps(x, w_qkv, b_qkv, w_proj, b_proj):
    bf = ml_dtypes.bfloat16
    in_maps = []
    for c in range(N_CORES):
        b = c // 2
        hg = c % 2
        cols = slice(DQ * hg, DQ * (hg + 1))
        w_q = w_qkv[:, 0:DIM][:, cols]
        w_k = w_qkv[:, DIM:2 * DIM][:, cols]
        w_v = w_qkv[:, 2 * DIM:3 * DIM][:, cols]
        b_s = np.ascontiguousarray(np.concatenate(
            [b_qkv[0:DIM][cols], b_qkv[DIM:2 * DIM][cols],
             b_qkv[2 * DIM:3 * DIM][cols]]))
        in_maps.append({
            "x_t": np.ascontiguousarray(x[b].T).astype(bf),
            "w_q_h": np.ascontiguousarray(
                w_q.reshape(KB_C, 128, 4, 128).transpose(1, 2, 0, 3)
            ).astype(bf),
            "w_k_h": np.ascontiguousarray(
                w_k.reshape(KB_C, 128, 4, 128).transpose(1, 2, 0, 3)
            ).astype(bf),
            "w_v_h": np.ascontiguousarray(
                w_v.reshape(KB_C, 128, DQ).transpose(1, 0, 2)).astype(bf),
            "w_p_h": np.ascontiguousarray(
                w_proj[DQ * hg:DQ * (hg + 1), :].reshape(4, 128, DIM)
                .transpose(1, 0, 2)).astype(bf),
            "b_qkv_s": b_s,
            "b_proj_h": (b_proj * 0.5).astype(np.float32),
        })
    return in_maps


def kernel(x, w_qkv, b_qkv, w_proj, b_proj):
    from concourse.bass_utils import run_bass_kernel_spmd

    x = np.asarray(x, dtype=np.float32)
    w_qkv = np.asarray(w_qkv, dtype=np.float32)
    b_qkv = np.asarray(b_qkv, dtype=np.float32)
    w_proj = np.asarray(w_proj, dtype=np.float32)
    b_proj = np.asarray(b_proj, dtype=np.float32)

    nc = _get_nc()
    in_maps = _make_in_maps(x, w_qkv, b_qkv, w_proj, b_proj)
    _CACHE["in_maps"] = in_maps

    res = run_bass_kernel_spmd(nc, in_maps, core_ids=list(range(N_CORES)))

    full = np.empty((B, T, DIM), dtype=np.float32)
    for c in range(N_CORES):
        b = c // 2
        p = c % 2
        o = np.asarray(res.results[c]["out"]).astype(np.float32)
        for qb in range(4):
            full[b, 512 * qb + 256 * p:512 * qb + 256 * (p + 1), :] = \
                o[256 * qb:256 * (qb + 1), :]
    return full


# revision 21
# speedup vs baseline: 1.1117x; 1.1117x over previous
"""Multi-head attention block on 8 TRN2 NeuronCores.

Sharding: core c -> (batch b = c//2, head-group hg = c%2).
Each core computes QKV projections for its 8 heads over its batch,
attention (bf16 QK^T with row-tiled concurrent head pairs, exp on ACT,
bf16 A@V with a col-packed ones-matmul producing replicated row-sums),
and a bf16 output projection of its head-group's channels. Pairs of
cores (same batch) combine partial projections with ReduceScatter
collectives; the host concatenates the 8 per-core output shards.

v2 schedule: single flat 256-task loop (4 q-blocks x 4 head-pairs x
16 k-blocks) with the exp pipeline as the pacing engine. All
projection work (K/V/Q/proj) is emitted through a per-q-block service
queue with just-in-time need-by indices so PE fills the exp-shadow
without starving ACT. Prologue uses batched priority DMAs (critical
1.5MB first) so the first exp fires ~10us in. Norms are pair-combined
(one reciprocal per head-pair) and chunked; qb3's ReduceScatter is
chunked per 128-row block to shorten the serial tail.
"""

import sys

if "/opt/trn_rl_repo" not in sys.path:
    sys.path.insert(0, "/opt/trn_rl_repo")

import numpy as np
import ml_dtypes

N_CORES = 8
B, T, DIM = 4, 2048, 1024
H_TOT, HD = 16, 64
HPC = H_TOT // 2          # heads per core (2 head-groups)
DQ = HPC * HD             # 512: per-core q/k/v width
SCALE = HD ** -0.5
KB_T = T // 128           # 16 k-blocks over sequence
KB_C = DIM // 128         # 8 k-blocks over channel dim

_CACHE = {}


def _build():
    import concourse.bass as bass
    import concourse.tile as tile
    from concourse import bacc, mybir

    F32 = mybir.dt.float32
    BF16 = mybir.dt.bfloat16
    AF = mybir.ActivationFunctionType

    nc = bacc.Bacc("TRN2", target_bir_lowering=False, debug=False,
                   num_devices=N_CORES)

    x_t = nc.dram_tensor("x_t", [DIM, T], BF16, kind="ExternalInput").ap()
    w_q_h = nc.dram_tensor("w_q_h", [128, 4, KB_C, 128], BF16,
                           kind="ExternalInput").ap()
    w_k_h = nc.dram_tensor("w_k_h", [128, 4, KB_C, 128], BF16,
                           kind="ExternalInput").ap()
    w_v_h = nc.dram_tensor("w_v_h", [128, KB_C, DQ], BF16,
                           kind="ExternalInput").ap()
    w_p_h = nc.dram_tensor("w_p_h", [128, 4, DIM], BF16,
                           kind="ExternalInput").ap()
    b_qkv = nc.dram_tensor("b_qkv_s", [3 * DQ], F32, kind="ExternalInput").ap()
    b_proj = nc.dram_tensor("b_proj_h", [DIM], F32, kind="ExternalInput").ap()
    out = nc.dram_tensor("out", [T // 2, DIM], BF16, kind="ExternalOutput").ap()
    # per-column-half planes so chunked ReduceScatter inputs stay contiguous
    partial_c = [nc.dram_tensor(f"partial{cb}", [T, DIM // 2], BF16).ap()
                 for cb in range(2)]
    rs_out_c = [nc.dram_tensor(f"rs_out{cb}", [T // 2, DIM // 2], BF16).ap()
                for cb in range(2)]
    partial_f = nc.dram_tensor("partial_f", [512, DIM], BF16).ap()
    rs_out_f = nc.dram_tensor("rs_out_f", [256, DIM], BF16).ap()

    groups = [[0, 1], [2, 3], [4, 5], [6, 7]]

    def bcast_ap(src_ap, parts):
        # partition-broadcast read of a 1-D DRAM row
        return bass.AP(tensor=src_ap.tensor, offset=src_ap.offset,
                       ap=[[0, parts]] + list(src_ap.ap))

    with tile.TileContext(nc) as tc:
        with (
            tc.tile_pool(name="persist", bufs=1) as pp,
        ):
            k_sb = pp.tile([128, 4, T], BF16)
            v_sb = pp.tile([128, KB_T, HPC, 2 * HD], BF16)
            wq_sb = pp.tile([128, 4, KB_C, 128], BF16)
            wk_c = pp.tile([128, 4, KB_C, 128], BF16)
            wv_c = pp.tile([128, KB_C, DQ], BF16)
            x_sb = pp.tile([128, KB_C, T], BF16)
            bqkv_sb = pp.tile([128, 12], F32)
            bv_sb = pp.tile([128, DQ], F32)

            nc.vector.memset(v_sb[:, :, :, HD:2 * HD], 1.0)

            with (
                tc.tile_pool(name="zb", bufs=1) as zb,
                tc.tile_pool(name="qsl", bufs=2) as qslp,
                tc.tile_pool(name="zpool", bufs=3) as zpool,
                tc.tile_pool(name="apool", bufs=5) as apool,
                tc.tile_pool(name="small", bufs=8) as small,
                tc.tile_pool(name="opool", bufs=4) as opool,
                tc.tile_pool(name="psS", bufs=2, space="PSUM") as pss,
                tc.tile_pool(name="psZ", bufs=2, space="PSUM") as psz,
                tc.tile_pool(name="psP", bufs=2, space="PSUM") as psp,
            ):
                wp_sb = zb.tile([128, 4, DIM], BF16)
                bp_sb = zb.tile([128, DIM], F32)
                warm1 = zb.tile([1, 4], F32)

                q_tiles = {}
                z_tiles = {}
                s_tiles = {}
                z_ps = {}
                zv_t = {}
                zs_t = {}
                qproj_ps = {}

                # ---------- emit helpers -------------------------------
                def emit_qproj_half(qb, m, h):
                    key = (qb, m)
                    if key not in qproj_ps:
                        if qb not in q_tiles:
                            q_tiles[qb] = qslp.tile([128, 4, 512], BF16,
                                                    tag="q", name=f"qt{qb}")
                        qproj_ps[key] = psp.tile([128, 512], F32, tag="pj",
                                                 name=f"qp{qb}_{m}")
                    ps = qproj_ps[key]
                    for kb in range(4 * h, 4 * h + 4):
                        nc.tensor.matmul(
                            ps[:],
                            wq_sb[:, m, kb, :],
                            x_sb[:, kb, 512 * qb:512 * (qb + 1)],
                            start=(kb == 0), stop=(kb == KB_C - 1))
                    if h == 1:
                        nc.vector.tensor_scalar_add(
                            out=q_tiles[qb][:, m, :],
                            in0=ps[:],
                            scalar1=bqkv_sb[:, m:m + 1])
                        del qproj_ps[key]

                def emit_k_half(m, half):
                    tcol = 512 * half
                    psx = psp.tile([128, 512], F32, tag="pj",
                                   name=f"kp{m}_{half}")
                    for kb in range(KB_C):
                        nc.tensor.matmul(
                            psx[:],
                            wk_c[:, m, kb, :],
                            x_sb[:, kb, tcol:tcol + 512],
                            start=(kb == 0), stop=(kb == KB_C - 1))
                    nc.vector.tensor_scalar_add(
                        out=k_sb[:, m, tcol:tcol + 512],
                        in0=psx[:], scalar1=bqkv_sb[:, 4 + m:5 + m])

                def emit_v_unit(tb):
                    ps = psp.tile([128, DQ], F32, tag="pj", name=f"vps{tb}")
                    for kb in range(KB_C):
                        nc.tensor.matmul(
                            ps[:],
                            x_sb[:, kb, 128 * tb:128 * (tb + 1)],
                            wv_c[:, kb, :],
                            start=(kb == 0), stop=(kb == KB_C - 1))
                    nc.vector.tensor_add(
                        v_sb[:, tb, :, 0:HD],
                        ps[:].rearrange("p (h d) -> p h d", h=HPC),
                        bv_sb[:].rearrange("p (h d) -> p h d", h=HPC))

                def emit_qk_pair(qb, hp, kb, s):
                    qt = q_tiles[qb]
                    kc = 128 * kb
                    nc.tensor.matmul(
                        s[:, 0, :],
                        k_sb[0:64, hp, kc:kc + 128],
                        qt[0:64, hp, :],
                        start=True, stop=True)
                    nc.tensor.matmul(
                        s[:, 1, :],
                        k_sb[64:128, hp, kc:kc + 128],
                        qt[64:128, hp, :],
                        start=True, stop=True)

                def emit_av_pair(hp, kb, z0, z1, a):
                    st = (kb == 0)
                    sp = (kb == KB_T - 1)
                    nc.tensor.matmul(
                        z0[:], v_sb[:, kb, 2 * hp, :],
                        a[:, 0, :], start=st, stop=sp)
                    nc.tensor.matmul(
                        z1[:], v_sb[:, kb, 2 * hp + 1, :],
                        a[:, 1, :], start=st, stop=sp)

                def emit_zvzs(qb, hp):
                    # split accumulators into a value-pair tile and a
                    # sums-pair tile (head 2hp -> parts 0:64, 2hp+1 -> 64:128)
                    zv = small.tile([128, 512], F32, tag="zv",
                                    name=f"zv{qb}_{hp}")
                    zs = small.tile([128, 512], F32, tag="zs",
                                    name=f"zs{qb}_{hp}")
                    nc.vector.tensor_copy(zv[0:64, :], z_ps[2 * hp][0:64, :])
                    nc.vector.tensor_copy(zv[64:128, :],
                                          z_ps[2 * hp + 1][0:64, :])
                    nc.vector.tensor_copy(zs[0:64, :],
                                          z_ps[2 * hp][64:128, :])
                    nc.vector.tensor_copy(zs[64:128, :],
                                          z_ps[2 * hp + 1][64:128, :])
                    zv_t[hp] = zv
                    zs_t[hp] = zs

                def emit_norm_chunk(qb, hp, ch):
                    # one 256-col chunk of the pair-combined normalize
                    c0, c1 = 256 * ch, 256 * (ch + 1)
                    rinv = small.tile([128, 256], F32, tag="rinv",
                                      name=f"ri{qb}_{hp}_{ch}")
                    nc.vector.reciprocal(rinv[:], zs_t[hp][:, c0:c1])
                    nc.gpsimd.tensor_mul(
                        z_tiles[qb][:, hp, c0:c1],
                        zv_t[hp][:, c0:c1], rinv[:])

                def emit_proj_unit(qb, tb4, cb):
                    t0 = 512 * qb + 128 * tb4
                    zt = z_tiles[qb]
                    ppj = psp.tile([128, 512], F32, tag="pj",
                                   name=f"pj{qb}_{tb4}_{cb}")
                    for m in range(4):
                        nc.tensor.matmul(
                            ppj[:],
                            zt[:, m, 128 * tb4:128 * (tb4 + 1)],
                            wp_sb[:, m, 512 * cb:512 * (cb + 1)],
                            start=(m == 0), stop=(m == 3))
                    o = opool.tile([128, 512], BF16, tag="o")
                    nc.vector.tensor_add(
                        o[:], ppj[:], bp_sb[:, 512 * cb:512 * (cb + 1)])
                    if qb == 3:
                        nc.sync.dma_start(
                            out=partial_f[128 * tb4:128 * (tb4 + 1),
                                          512 * cb:512 * (cb + 1)],
                            in_=o[:])
                    else:
                        nc.sync.dma_start(
                            out=partial_c[cb][t0:t0 + 128, :],
                            in_=o[:])

                def emit_rs(qb, cb):
                    # column-half chunks: flat split of [512 rows, 512 cols]
                    # hands member p the contiguous 256-row half it owns
                    r0, r1 = 512 * qb, 512 * (qb + 1)
                    o0, o1 = 256 * qb, 256 * (qb + 1)
                    nc.gpsimd.collective_compute(
                        "ReduceScatter",
                        mybir.AluOpType.add,
                        ins=[partial_c[cb][r0:r1, :]],
                        outs=[rs_out_c[cb][o0:o1, :]],
                        replica_groups=groups,
                    )
                    nc.sync.dma_start(
                        out=out[o0:o1, 512 * cb:512 * (cb + 1)],
                        in_=rs_out_c[cb][o0:o1, :])

                def emit_rs_full():
                    nc.gpsimd.collective_compute(
                        "ReduceScatter",
                        mybir.AluOpType.add,
                        ins=[partial_f[:]],
                        outs=[rs_out_f[:]],
                        replica_groups=groups,
                    )
                    nc.sync.dma_start(out=out[768:1024, :], in_=rs_out_f[:])

                # ---------- prologue -----------------------------------
                # warm the exp table set while DMAs stream
                nc.vector.memset(warm1[:], 0.0)
                nc.scalar.activation(out=warm1[:], in_=warm1[:],
                                     func=AF.Exp, scale=1.0)

                nc.sync.dma_start(out=bqkv_sb,
                                  in_=b_qkv.rearrange("(m p) -> p m", p=128))
                nc.sync.dma_start(out=bv_sb,
                                  in_=bcast_ap(b_qkv[2 * DQ:3 * DQ], 128))
                x_r = x_t.rearrange("(kb p) t -> p kb t", p=128)
                nc.sync.dma_start(out=x_sb[:, 0:4, 0:512],
                                  in_=x_r[:, 0:4, 0:512])
                nc.sync.dma_start(out=x_sb[:, 4:8, 0:512],
                                  in_=x_r[:, 4:8, 0:512])
                nc.sync.dma_start(out=wk_c[:, 0], in_=w_k_h[:, 0])
                nc.sync.dma_start(out=wq_sb[:, 0], in_=w_q_h[:, 0])
                nc.sync.dma_start(out=wv_c, in_=w_v_h)

                emit_k_half(0, 0)
                emit_qproj_half(0, 0, 0)
                emit_qproj_half(0, 0, 1)

                nc.sync.dma_start(out=x_sb[:, :, 512:1024],
                                  in_=x_r[:, :, 512:1024])
                nc.sync.dma_start(out=x_sb[:, :, 1024:1536],
                                  in_=x_r[:, :, 1024:1536])
                nc.sync.dma_start(out=x_sb[:, :, 1536:2048],
                                  in_=x_r[:, :, 1536:2048])
                nc.sync.dma_start(out=wk_c[:, 1:4], in_=w_k_h[:, 1:4])
                nc.sync.dma_start(out=wq_sb[:, 1:4], in_=w_q_h[:, 1:4])
                nc.sync.dma_start(out=wp_sb, in_=w_p_h)
                nc.sync.dma_start(out=bp_sb, in_=bcast_ap(b_proj[:], 128))

                # ---------- service queues -----------------------------
                def build_svc(qb):
                    svc = []
                    if qb == 0:
                        for tb in range(2, KB_T):
                            svc.append((max(0, tb - 1),
                                        lambda tb=tb: emit_v_unit(tb)))
                        for m in range(4):
                            for half in range(4):
                                if m == 0 and half == 0:
                                    continue
                                svc.append(
                                    (max(0, 16 * m + 4 * half - 2),
                                     lambda m=m, half=half:
                                     emit_k_half(m, half)))
                        for m in range(1, 4):
                            svc.append((16 * m - 4,
                                        lambda m=m: emit_qproj_half(0, m, 0)))
                            svc.append((16 * m - 3,
                                        lambda m=m: emit_qproj_half(0, m, 1)))
                        svc.append((56, lambda: emit_qproj_half(1, 0, 0)))
                        svc.append((58, lambda: emit_qproj_half(1, 0, 1)))
                    else:
                        for m in range(1, 4):
                            svc.append((16 * m - 4,
                                        lambda m=m:
                                        emit_qproj_half(qb, m, 0)))
                            svc.append((16 * m - 3,
                                        lambda m=m:
                                        emit_qproj_half(qb, m, 1)))
                        if qb < 3:
                            svc.append((56,
                                        lambda: emit_qproj_half(qb + 1, 0, 0)))
                            svc.append((58,
                                        lambda: emit_qproj_half(qb + 1, 0, 1)))
                        for u in range(8):
                            cb, tb4 = u // 4, u % 4
                            svc.append(
                                (4 + 6 * u,
                                 lambda tb4=tb4, cb=cb:
                                 emit_proj_unit(qb - 1, tb4, cb)))
                        svc.append((26, lambda: emit_rs(qb - 1, 0)))
                        svc.append((52, lambda: emit_rs(qb - 1, 1)))
                    svc.sort(key=lambda x: x[0])
                    return svc

                # ---------- main loop ----------------------------------
                NT = 256
                z_tiles[0] = zpool.tile([128, 4, 512], BF16, tag="z",
                                        name="zt0")
                s_tiles[0] = pss.tile([128, 2, 512], F32, tag="s", name="s0")
                emit_qk_pair(0, 0, 0, s_tiles[0])
                emit_v_unit(0)
                emit_v_unit(1)
                svc = build_svc(0)
                svc_i = 0

                for g in range(NT):
                    qb, idx = g // 64, g % 64
                    hp, kb = idx // 16, idx % 16

                    if idx == 0 and qb > 0:
                        svc = build_svc(qb)
                        svc_i = 0

                    # next task's QK (cross-boundary safe)
                    if g + 1 < NT:
                        nqb = (g + 1) // 64
                        if nqb != qb:
                            z_tiles[nqb] = zpool.tile([128, 4, 512], BF16,
                                                      tag="z", name=f"zt{nqb}")
                        nhp, nkb = ((g + 1) % 64) // 16, ((g + 1) % 64) % 16
                        s_tiles[g + 1] = pss.tile([128, 2, 512], F32, tag="s",
                                                  name=f"s{g + 1}")
                        emit_qk_pair(nqb, nhp, nkb, s_tiles[g + 1])

                    # exp of current task
                    a = apool.tile([128, 2, 512], BF16, tag="a",
                                   name=f"a{g}")
                    nc.scalar.activation(out=a[:], in_=s_tiles[g][:],
                                         func=AF.Exp, scale=SCALE)
                    del s_tiles[g]

                    # service work fills the exp shadow
                    while svc_i < len(svc) and svc[svc_i][0] <= idx:
                        svc[svc_i][1]()
                        svc_i += 1

                    # A@V accumulation
                    if kb == 0:
                        z_ps[2 * hp] = psz.tile([128, 512], F32, tag="z",
                                                name=f"zp{qb}_{2 * hp}")
                        z_ps[2 * hp + 1] = psz.tile(
                            [128, 512], F32, tag="z",
                            name=f"zp{qb}_{2 * hp + 1}")
                    emit_av_pair(hp, kb, z_ps[2 * hp], z_ps[2 * hp + 1], a)

                    # norms of the previous pair, chunked over 2 tasks
                    if hp >= 1 and kb in (2, 3):
                        emit_norm_chunk(qb, hp - 1, kb - 2)
                    if kb == KB_T - 1:
                        emit_zvzs(qb, hp)
                        if hp == 3:
                            emit_norm_chunk(qb, 3, 0)
                            emit_norm_chunk(qb, 3, 1)

                # ---------- tail: qb3 proj + one full-width RS ---------
                for tb4 in range(4):
                    emit_proj_unit(3, tb4, 0)
                    emit_proj_unit(3, tb4, 1)
                emit_rs_full()

    nc.compile()
    return nc


def _get_nc():
    if "nc" not in _CACHE:
        _CACHE["nc"] = _build()
    return _CACHE["nc"]


def _make_in_maps(x, w_qkv, b_qkv, w_proj, b_proj):
    bf = ml_dtypes.bfloat16
    in_maps = []
    for c in range(N_CORES):
        b = c // 2
        hg = c % 2
        cols = slice(DQ * hg, DQ * (hg + 1))
        w_q = w_qkv[:, 0:DIM][:, cols]
        w_k = w_qkv[:, DIM:2 * DIM][:, cols]
        w_v = w_qkv[:, 2 * DIM:3 * DIM][:, cols]
        b_s = np.ascontiguousarray(np.concatenate(
            [b_qkv[0:DIM][cols], b_qkv[DIM:2 * DIM][cols],
             b_qkv[2 * DIM:3 * DIM][cols]]))
        in_maps.append({
            "x_t": np.ascontiguousarray(x[b].T).astype(bf),
            "w_q_h": np.ascontiguousarray(
                w_q.reshape(KB_C, 128, 4, 128).transpose(1, 2, 0, 3)
            ).astype(bf),
            "w_k_h": np.ascontiguousarray(
                w_k.reshape(KB_C, 128, 4, 128).transpose(1, 2, 0, 3)
            ).astype(bf),
            "w_v_h": np.ascontiguousarray(
                w_v.reshape(KB_C, 128, DQ).transpose(1, 0, 2)).astype(bf),
            "w_p_h": np.ascontiguousarray(
                w_proj[DQ * hg:DQ * (hg + 1), :].reshape(4, 128, DIM)
                .transpose(1, 0, 2)).astype(bf),
            "b_qkv_s": b_s,
            "b_proj_h": (b_proj * 0.5).astype(np.float32),
        })
    return in_maps


def kernel(x, w_qkv, b_qkv, w_proj, b_proj):
    from concourse.bass_utils import run_bass_kernel_spmd

    x = np.asarray(x, dtype=np.float32)
    w_qkv = np.asarray(w_qkv, dtype=np.float32)
    b_qkv = np.asarray(b_qkv, dtype=np.float32)
    w_proj = np.asarray(w_proj, dtype=np.float32)
    b_proj = np.asarray(b_proj, dtype=np.float32)

    nc = _get_nc()
    in_maps = _make_in_maps(x, w_qkv, b_qkv, w_proj, b_proj)
    _CACHE["in_maps"] = in_maps

    res = run_bass_kernel_spmd(nc, in_maps, core_ids=list(range(N_CORES)))

    full = np.empty((B, T, DIM), dtype=np.float32)
    for c in range(N_CORES):
        b = c // 2
        p = c % 2
        o = np.asarray(res.results[c]["out"]).astype(np.float32)
        for qb in range(4):
            full[b, 512 * qb + 256 * p:512 * qb + 256 * (p + 1), :] = \
                o[256 * qb:256 * (qb + 1), :]
    return full


# revision 23
# speedup vs baseline: 1.1194x; 1.0069x over previous
"""Multi-head attention block on 8 TRN2 NeuronCores.

Sharding: core c -> (batch b = c//2, head-group hg = c%2).
Each core computes QKV projections for its 8 heads over its batch,
attention (bf16 QK^T with row-tiled concurrent head pairs, exp on ACT,
bf16 A@V with a col-packed ones-matmul producing replicated row-sums),
and a bf16 output projection of its head-group's channels. Pairs of
cores (same batch) combine partial projections with ReduceScatter
collectives; the host concatenates the 8 per-core output shards.

v2 schedule: single flat 256-task loop (4 q-blocks x 4 head-pairs x
16 k-blocks) with the exp pipeline as the pacing engine. All
projection work (K/V/Q/proj) is emitted through a per-q-block service
queue with just-in-time need-by indices so PE fills the exp-shadow
without starving ACT. Prologue uses batched priority DMAs (critical
1.5MB first) so the first exp fires ~10us in. Norms are pair-combined
(one reciprocal per head-pair) and chunked; qb3's ReduceScatter is
chunked per 128-row block to shorten the serial tail.
"""

import sys

if "/opt/trn_rl_repo" not in sys.path:
    sys.path.insert(0, "/opt/trn_rl_repo")

import numpy as np
import ml_dtypes

N_CORES = 8
B, T, DIM = 4, 2048, 1024
H_TOT, HD = 16, 64
HPC = H_TOT // 2          # heads per core (2 head-groups)
DQ = HPC * HD             # 512: per-core q/k/v width
SCALE = HD ** -0.5
KB_T = T // 128           # 16 k-blocks over sequence
KB_C = DIM // 128         # 8 k-blocks over channel dim

_CACHE = {}


def _build():
    import concourse.bass as bass
    import concourse.tile as tile
    from concourse import bacc, mybir

    F32 = mybir.dt.float32
    BF16 = mybir.dt.bfloat16
    AF = mybir.ActivationFunctionType

    nc = bacc.Bacc("TRN2", target_bir_lowering=False, debug=False,
                   num_devices=N_CORES)

    x_t = nc.dram_tensor("x_t", [DIM, T], BF16, kind="ExternalInput").ap()
    w_q_h = nc.dram_tensor("w_q_h", [128, 4, KB_C, 128], BF16,
                           kind="ExternalInput").ap()
    w_k_h = nc.dram_tensor("w_k_h", [128, 4, KB_C, 128], BF16,
                           kind="ExternalInput").ap()
    w_v_h = nc.dram_tensor("w_v_h", [128, KB_C, DQ], BF16,
                           kind="ExternalInput").ap()
    w_p_h = nc.dram_tensor("w_p_h", [128, 4, DIM], BF16,
                           kind="ExternalInput").ap()
    b_qkv = nc.dram_tensor("b_qkv_s", [3 * DQ], F32, kind="ExternalInput").ap()
    b_proj = nc.dram_tensor("b_proj_h", [DIM], F32, kind="ExternalInput").ap()
    out = nc.dram_tensor("out", [T // 2, DIM], BF16, kind="ExternalOutput").ap()
    # per-column-half planes so chunked ReduceScatter inputs stay contiguous
    partial_c = [nc.dram_tensor(f"partial{cb}", [T, DIM // 2], BF16).ap()
                 for cb in range(2)]
    rs_out_c = [nc.dram_tensor(f"rs_out{cb}", [T // 2, DIM // 2], BF16).ap()
                for cb in range(2)]
    partial_f = nc.dram_tensor("partial_f", [512, DIM], BF16).ap()
    rs_out_f = nc.dram_tensor("rs_out_f", [256, DIM], BF16).ap()

    groups = [[0, 1], [2, 3], [4, 5], [6, 7]]

    def bcast_ap(src_ap, parts):
        # partition-broadcast read of a 1-D DRAM row
        return bass.AP(tensor=src_ap.tensor, offset=src_ap.offset,
                       ap=[[0, parts]] + list(src_ap.ap))

    with tile.TileContext(nc) as tc:
        with (
            tc.tile_pool(name="persist", bufs=1) as pp,
        ):
            k_sb = pp.tile([128, 4, T], BF16)
            v_sb = pp.tile([128, KB_T, HPC, 2 * HD], BF16)
            wq_sb = pp.tile([128, 4, KB_C, 128], BF16)
            wk_c = pp.tile([128, 4, KB_C, 128], BF16)
            wv_c = pp.tile([128, KB_C, DQ], BF16)
            x_sb = pp.tile([128, KB_C, T], BF16)
            bqkv_sb = pp.tile([128, 12], F32)
            bv_sb = pp.tile([128, DQ], F32)

            nc.vector.memset(v_sb[:, :, :, HD:2 * HD], 1.0)

            with (
                tc.tile_pool(name="zb", bufs=1) as zb,
                tc.tile_pool(name="qsl", bufs=2) as qslp,
                tc.tile_pool(name="zpool", bufs=3) as zpool,
                tc.tile_pool(name="apool", bufs=5) as apool,
                tc.tile_pool(name="small", bufs=8) as small,
                tc.tile_pool(name="opool", bufs=4) as opool,
                tc.tile_pool(name="psS", bufs=2, space="PSUM") as pss,
                tc.tile_pool(name="psZ", bufs=2, space="PSUM") as psz,
                tc.tile_pool(name="psP", bufs=2, space="PSUM") as psp,
            ):
                wp_sb = zb.tile([128, 4, DIM], BF16)
                bp_sb = zb.tile([128, DIM], F32)
                warm1 = zb.tile([1, 4], F32)

                q_tiles = {}
                z_tiles = {}
                s_tiles = {}
                z_ps = {}
                zv_t = {}
                zs_t = {}
                qproj_ps = {}

                # ---------- emit helpers -------------------------------
                def emit_qproj_half(qb, m, h):
                    key = (qb, m)
                    if key not in qproj_ps:
                        if qb not in q_tiles:
                            q_tiles[qb] = qslp.tile([128, 4, 512], BF16,
                                                    tag="q", name=f"qt{qb}")
                        qproj_ps[key] = psp.tile([128, 512], F32, tag="pj",
                                                 name=f"qp{qb}_{m}")
                    ps = qproj_ps[key]
                    for kb in range(4 * h, 4 * h + 4):
                        nc.tensor.matmul(
                            ps[:],
                            wq_sb[:, m, kb, :],
                            x_sb[:, kb, 512 * qb:512 * (qb + 1)],
                            start=(kb == 0), stop=(kb == KB_C - 1))
                    if h == 1:
                        nc.vector.tensor_scalar_add(
                            out=q_tiles[qb][:, m, :],
                            in0=ps[:],
                            scalar1=bqkv_sb[:, m:m + 1])
                        del qproj_ps[key]

                def emit_k_half(m, half):
                    tcol = 512 * half
                    psx = psp.tile([128, 512], F32, tag="pj",
                                   name=f"kp{m}_{half}")
                    for kb in range(KB_C):
                        nc.tensor.matmul(
                            psx[:],
                            wk_c[:, m, kb, :],
                            x_sb[:, kb, tcol:tcol + 512],
                            start=(kb == 0), stop=(kb == KB_C - 1))
                    nc.vector.tensor_scalar_add(
                        out=k_sb[:, m, tcol:tcol + 512],
                        in0=psx[:], scalar1=bqkv_sb[:, 4 + m:5 + m])

                def emit_v_unit(tb):
                    ps = psp.tile([128, DQ], F32, tag="pj", name=f"vps{tb}")
                    for kb in range(KB_C):
                        nc.tensor.matmul(
                            ps[:],
                            x_sb[:, kb, 128 * tb:128 * (tb + 1)],
                            wv_c[:, kb, :],
                            start=(kb == 0), stop=(kb == KB_C - 1))
                    nc.vector.tensor_add(
                        v_sb[:, tb, :, 0:HD],
                        ps[:].rearrange("p (h d) -> p h d", h=HPC),
                        bv_sb[:].rearrange("p (h d) -> p h d", h=HPC))

                def emit_qk_pair(qb, hp, kb, s):
                    qt = q_tiles[qb]
                    kc = 128 * kb
                    nc.tensor.matmul(
                        s[:, 0, :],
                        k_sb[0:64, hp, kc:kc + 128],
                        qt[0:64, hp, :],
                        start=True, stop=True)
                    nc.tensor.matmul(
                        s[:, 1, :],
                        k_sb[64:128, hp, kc:kc + 128],
                        qt[64:128, hp, :],
                        start=True, stop=True)

                def emit_av_pair(hp, kb, z0, z1, a):
                    st = (kb == 0)
                    sp = (kb == KB_T - 1)
                    nc.tensor.matmul(
                        z0[:], v_sb[:, kb, 2 * hp, :],
                        a[:, 0, :], start=st, stop=sp)
                    nc.tensor.matmul(
                        z1[:], v_sb[:, kb, 2 * hp + 1, :],
                        a[:, 1, :], start=st, stop=sp)

                def emit_zvzs(qb, hp):
                    # split accumulators into a value-pair tile and a
                    # sums-pair tile (head 2hp -> parts 0:64, 2hp+1 -> 64:128)
                    zv = small.tile([128, 512], F32, tag="zv",
                                    name=f"zv{qb}_{hp}")
                    zs = small.tile([128, 512], F32, tag="zs",
                                    name=f"zs{qb}_{hp}")
                    nc.vector.tensor_copy(zv[0:64, :], z_ps[2 * hp][0:64, :])
                    nc.vector.tensor_copy(zv[64:128, :],
                                          z_ps[2 * hp + 1][0:64, :])
                    nc.vector.tensor_copy(zs[0:64, :],
                                          z_ps[2 * hp][64:128, :])
                    nc.vector.tensor_copy(zs[64:128, :],
                                          z_ps[2 * hp + 1][64:128, :])
                    zv_t[hp] = zv
                    zs_t[hp] = zs

                def emit_norm_chunk(qb, hp, ch):
                    # one 256-col chunk of the pair-combined normalize
                    c0, c1 = 256 * ch, 256 * (ch + 1)
                    rinv = small.tile([128, 256], F32, tag="rinv",
                                      name=f"ri{qb}_{hp}_{ch}")
                    nc.vector.reciprocal(rinv[:], zs_t[hp][:, c0:c1])
                    nc.gpsimd.tensor_mul(
                        z_tiles[qb][:, hp, c0:c1],
                        zv_t[hp][:, c0:c1], rinv[:])

                def emit_proj_unit(qb, tb4, cb):
                    t0 = 512 * qb + 128 * tb4
                    zt = z_tiles[qb]
                    ppj = psp.tile([128, 512], F32, tag="pj",
                                   name=f"pj{qb}_{tb4}_{cb}")
                    for m in range(4):
                        nc.tensor.matmul(
                            ppj[:],
                            zt[:, m, 128 * tb4:128 * (tb4 + 1)],
                            wp_sb[:, m, 512 * cb:512 * (cb + 1)],
                            start=(m == 0), stop=(m == 3))
                    o = opool.tile([128, 512], BF16, tag="o")
                    nc.vector.tensor_add(
                        o[:], ppj[:], bp_sb[:, 512 * cb:512 * (cb + 1)])
                    if qb == 3:
                        nc.sync.dma_start(
                            out=partial_f[128 * tb4:128 * (tb4 + 1),
                                          512 * cb:512 * (cb + 1)],
                            in_=o[:])
                    else:
                        nc.sync.dma_start(
                            out=partial_c[cb][t0:t0 + 128, :],
                            in_=o[:])

                def emit_rs(qb, cb):
                    # column-half chunks: flat split of [512 rows, 512 cols]
                    # hands member p the contiguous 256-row half it owns
                    r0, r1 = 512 * qb, 512 * (qb + 1)
                    o0, o1 = 256 * qb, 256 * (qb + 1)
                    nc.gpsimd.collective_compute(
                        "ReduceScatter",
                        mybir.AluOpType.add,
                        ins=[partial_c[cb][r0:r1, :]],
                        outs=[rs_out_c[cb][o0:o1, :]],
                        replica_groups=groups,
                    )
                    nc.sync.dma_start(
                        out=out[o0:o1, 512 * cb:512 * (cb + 1)],
                        in_=rs_out_c[cb][o0:o1, :])

                def emit_rs_full():
                    nc.gpsimd.collective_compute(
                        "ReduceScatter",
                        mybir.AluOpType.add,
                        ins=[partial_f[:]],
                        outs=[rs_out_f[:]],
                        replica_groups=groups,
                    )
                    nc.sync.dma_start(out=out[768:1024, :], in_=rs_out_f[:])

                # ---------- prologue -----------------------------------
                # warm the exp table set while DMAs stream
                nc.vector.memset(warm1[:], 0.0)
                nc.scalar.activation(out=warm1[:], in_=warm1[:],
                                     func=AF.Exp, scale=1.0)

                # HAM warmup: keep PE busy through the DMA wait so the
                # first real chains run at 2.4GHz instead of 1.2
                warmm = zb.tile([128, 512], BF16)
                nc.vector.memset(warmm[:], 0.25)
                wps = psp.tile([128, 512], F32, tag="pj", name="warmps")
                for i in range(12):
                    nc.tensor.matmul(wps[:], warmm[:, 0:128], warmm[:],
                                     start=(i == 0), stop=(i == 11))

                x_r = x_t.rearrange("(kb p) t -> p kb t", p=128)
                nc.sync.dma_start(out=x_sb[:, 0:4, 0:512],
                                  in_=x_r[:, 0:4, 0:512])
                nc.sync.dma_start(out=wk_c[:, 0], in_=w_k_h[:, 0])
                nc.sync.dma_start(out=x_sb[:, 4:8, 0:512],
                                  in_=x_r[:, 4:8, 0:512])
                nc.sync.dma_start(out=wq_sb[:, 0], in_=w_q_h[:, 0])
                nc.sync.dma_start(out=bqkv_sb,
                                  in_=b_qkv.rearrange("(m p) -> p m", p=128))
                nc.sync.dma_start(out=bv_sb,
                                  in_=bcast_ap(b_qkv[2 * DQ:3 * DQ], 128))
                nc.sync.dma_start(out=wv_c, in_=w_v_h)

                emit_k_half(0, 0)
                emit_qproj_half(0, 0, 0)
                emit_qproj_half(0, 0, 1)

                nc.sync.dma_start(out=x_sb[:, :, 512:1024],
                                  in_=x_r[:, :, 512:1024])
                nc.sync.dma_start(out=x_sb[:, :, 1024:1536],
                                  in_=x_r[:, :, 1024:1536])
                nc.sync.dma_start(out=x_sb[:, :, 1536:2048],
                                  in_=x_r[:, :, 1536:2048])
                nc.sync.dma_start(out=wk_c[:, 1:4], in_=w_k_h[:, 1:4])
                nc.sync.dma_start(out=wq_sb[:, 1:4], in_=w_q_h[:, 1:4])
                nc.sync.dma_start(out=wp_sb, in_=w_p_h)
                nc.sync.dma_start(out=bp_sb, in_=bcast_ap(b_proj[:], 128))

                # ---------- service queues -----------------------------
                def build_svc(qb):
                    svc = []
                    if qb == 0:
                        for tb in range(2, KB_T):
                            svc.append((max(0, tb - 1),
                                        lambda tb=tb: emit_v_unit(tb)))
                        for m in range(4):
                            for half in range(4):
                                if m == 0 and half == 0:
                                    continue
                                svc.append(
                                    (max(0, 16 * m + 4 * half - 2),
                                     lambda m=m, half=half:
                                     emit_k_half(m, half)))
                        for m in range(1, 4):
                            svc.append((16 * m - 4,
                                        lambda m=m: emit_qproj_half(0, m, 0)))
                            svc.append((16 * m - 3,
                                        lambda m=m: emit_qproj_half(0, m, 1)))
                        svc.append((56, lambda: emit_qproj_half(1, 0, 0)))
                        svc.append((58, lambda: emit_qproj_half(1, 0, 1)))
                    else:
                        for m in range(1, 4):
                            svc.append((16 * m - 4,
                                        lambda m=m:
                                        emit_qproj_half(qb, m, 0)))
                            svc.append((16 * m - 3,
                                        lambda m=m:
                                        emit_qproj_half(qb, m, 1)))
                        if qb < 3:
                            svc.append((56,
                                        lambda: emit_qproj_half(qb + 1, 0, 0)))
                            svc.append((58,
                                        lambda: emit_qproj_half(qb + 1, 0, 1)))
                        for u in range(8):
                            cb, tb4 = u // 4, u % 4
                            svc.append(
                                (4 + 6 * u,
                                 lambda tb4=tb4, cb=cb:
                                 emit_proj_unit(qb - 1, tb4, cb)))
                        svc.append((26, lambda: emit_rs(qb - 1, 0)))
                        svc.append((52, lambda: emit_rs(qb - 1, 1)))
                    svc.sort(key=lambda x: x[0])
                    return svc

                # ---------- main loop ----------------------------------
                NT = 256
                z_tiles[0] = zpool.tile([128, 4, 512], BF16, tag="z",
                                        name="zt0")
                s_tiles[0] = pss.tile([128, 2, 512], F32, tag="s", name="s0")
                emit_qk_pair(0, 0, 0, s_tiles[0])
                emit_v_unit(0)
                emit_v_unit(1)
                svc = build_svc(0)
                svc_i = 0

                for g in range(NT):
                    qb, idx = g // 64, g % 64
                    hp, kb = idx // 16, idx % 16

                    if idx == 0 and qb > 0:
                        svc = build_svc(qb)
                        svc_i = 0

                    # next task's QK (cross-boundary safe)
                    if g + 1 < NT:
                        nqb = (g + 1) // 64
                        if nqb != qb:
                            z_tiles[nqb] = zpool.tile([128, 4, 512], BF16,
                                                      tag="z", name=f"zt{nqb}")
                        nhp, nkb = ((g + 1) % 64) // 16, ((g + 1) % 64) % 16
                        s_tiles[g + 1] = pss.tile([128, 2, 512], F32, tag="s",
                                                  name=f"s{g + 1}")
                        emit_qk_pair(nqb, nhp, nkb, s_tiles[g + 1])

                    # exp of current task
                    a = apool.tile([128, 2, 512], BF16, tag="a",
                                   name=f"a{g}")
                    nc.scalar.activation(out=a[:], in_=s_tiles[g][:],
                                         func=AF.Exp, scale=SCALE)
                    del s_tiles[g]

                    # service work fills the exp shadow
                    while svc_i < len(svc) and svc[svc_i][0] <= idx:
                        svc[svc_i][1]()
                        svc_i += 1

                    # A@V accumulation
                    if kb == 0:
                        z_ps[2 * hp] = psz.tile([128, 512], F32, tag="z",
                                                name=f"zp{qb}_{2 * hp}")
                        z_ps[2 * hp + 1] = psz.tile(
                            [128, 512], F32, tag="z",
                            name=f"zp{qb}_{2 * hp + 1}")
                    emit_av_pair(hp, kb, z_ps[2 * hp], z_ps[2 * hp + 1], a)

                    # norms of the previous pair, chunked over 2 tasks
                    if hp >= 1 and kb in (2, 3):
                        emit_norm_chunk(qb, hp - 1, kb - 2)
                    if kb == KB_T - 1:
                        emit_zvzs(qb, hp)
                        if hp == 3:
                            emit_norm_chunk(qb, 3, 0)
                            emit_norm_chunk(qb, 3, 1)

                # ---------- tail: qb3 proj + one full-width RS ---------
                for tb4 in range(4):
                    emit_proj_unit(3, tb4, 0)
                    emit_proj_unit(3, tb4, 1)
                emit_rs_full()

    nc.compile()
    return nc


def _get_nc():
    if "nc" not in _CACHE:
        _CACHE["nc"] = _build()
    return _CACHE["nc"]


def _make_in_maps(x, w_qkv, b_qkv, w_proj, b_proj):
    bf = ml_dtypes.bfloat16
    in_maps = []
    for c in range(N_CORES):
        b = c // 2
        hg = c % 2
        cols = slice(DQ * hg, DQ * (hg + 1))
        w_q = w_qkv[:, 0:DIM][:, cols]
        w_k = w_qkv[:, DIM:2 * DIM][:, cols]
        w_v = w_qkv[:, 2 * DIM:3 * DIM][:, cols]
        b_s = np.ascontiguousarray(np.concatenate(
            [b_qkv[0:DIM][cols], b_qkv[DIM:2 * DIM][cols],
             b_qkv[2 * DIM:3 * DIM][cols]]))
        in_maps.append({
            "x_t": np.ascontiguousarray(x[b].T).astype(bf),
            "w_q_h": np.ascontiguousarray(
                w_q.reshape(KB_C, 128, 4, 128).transpose(1, 2, 0, 3)
            ).astype(bf),
            "w_k_h": np.ascontiguousarray(
                w_k.reshape(KB_C, 128, 4, 128).transpose(1, 2, 0, 3)
            ).astype(bf),
            "w_v_h": np.ascontiguousarray(
                w_v.reshape(KB_C, 128, DQ).transpose(1, 0, 2)).astype(bf),
            "w_p_h": np.ascontiguousarray(
                w_proj[DQ * hg:DQ * (hg + 1), :].reshape(4, 128, DIM)
                .transpose(1, 0, 2)).astype(bf),
            "b_qkv_s": b_s,
            "b_proj_h": (b_proj * 0.5).astype(np.float32),
        })
    return in_maps


def kernel(x, w_qkv, b_qkv, w_proj, b_proj):
    from concourse.bass_utils import run_bass_kernel_spmd

    x = np.asarray(x, dtype=np.float32)
    w_qkv = np.asarray(w_qkv, dtype=np.float32)
    b_qkv = np.asarray(b_qkv, dtype=np.float32)
    w_proj = np.asarray(w_proj, dtype=np.float32)
    b_proj = np.asarray(b_proj, dtype=np.float32)

    nc = _get_nc()
    in_maps = _make_in_maps(x, w_qkv, b_qkv, w_proj, b_proj)
    _CACHE["in_maps"] = in_maps

    res = run_bass_kernel_spmd(nc, in_maps, core_ids=list(range(N_CORES)))

    full = np.empty((B, T, DIM), dtype=np.float32)
    for c in range(N_CORES):
        b = c // 2
        p = c % 2
        o = np.asarray(res.results[c]["out"]).astype(np.float32)
        for qb in range(4):
            full[b, 512 * qb + 256 * p:512 * qb + 256 * (p + 1), :] = \
                o[256 * qb:256 * (qb + 1), :]
    return full
